# revision 22
# baseline (speedup 1.0000x reference)
"""Trainium2 Bass kernel for nn_EnhancedCoordAtt (coordinate attention).

Strategy (pure data parallel, 8 cores x 4 samples, fp16 data path):
  - x [32,256,64,64] is converted to fp16 on the host (tolerance is 2e-2;
    fp16 keeps us ~1e-3) and sharded on batch; per core, per (sample,
    channel-block of 128) a [128, 4096] fp16 SBUF tile is streamed in.
    fp16 halves HBM traffic and doubles DVE tensor_tensor throughput
    (2x_1p packed mode; DVE reduce has no fast mode, so reduces are
    replaced by pairwise TT add trees which do run at 2x).
  - Pooling: row-half sums via a w-quarter add tree (4 levels + tiny pair
    op), column-half sums via a row-pair add tree (5 levels), both writing
    into a [128, 4, 128] y-tensor (concat([xh, xw]) layout with the
    full-mean channels folded into the weights host-side).
  - The dilated 3x3 conv is 12 accumulated PE matmuls into [8,128] PSUM;
    SiLU is done sigmoid-only (ACT sigmoid + fused DVE mul+reduce) so the
    ACT engine never swaps activation tables; SE gate on ACT.
  - Gate application: the h-gate sigmoid is fused with a broadcast
    expansion on the (otherwise idle) ACT engine into a full [128,64,64]
    fp16 tile so that both final multiplies have unit-stride operands and
    hit the DVE 2x mode; the w-gate broadcasts along a middle dim which
    is 2x-eligible directly.  out = (x * ahx) * aw_bcast, stored fp16 and
    upconverted to f32 on the host.
All shapes/constants hardcoded to the nn_EnhancedCoordAtt_78855599555233 spec.
"""

import numpy as np

N, C, H, W = 32, 256, 64, 64
MIP = 8
N_CORES = 8
S = N // N_CORES           # samples per core
HW = H * W
T = H + W                  # 128
BN_EPS = 1e-5

_CACHE = {}


def _legalize_waits(nc, mybir, max_keep=1):
    """walrus encodes at most one sync-wait on most compute/DMA ISA structs.
    Move excess waits onto standalone EventSemaphore (wait-only) instructions
    inserted immediately before the offender in its engine stream."""
    f = nc.m.functions[0]
    for blk in f.blocks:
        out, changed = [], False
        for inst in blk.instructions:
            si = inst.sync_info
            t = type(inst).__name__
            if (si is not None and len(si.on_wait) > max_keep
                    and t != "InstEventSemaphore"):
                waits = list(si.on_wait)
                for j, w in enumerate(waits[:-max_keep]):
                    ev = mybir.InstEventSemaphore(
                        name=f"{inst.name}_xw{j}", ins=[], outs=[])
                    ev.engine = inst.engine
                    ev.sync_info = mybir.SyncInfo(on_wait=[w], on_update=[])
                    out.append(ev)
                inst.sync_info = mybir.SyncInfo(
                    on_wait=waits[-max_keep:], on_update=list(si.on_update))
                changed = True
            out.append(inst)
        if changed:
            blk.instructions = out


def _build_program(legalize=True, sim_compat=False):
    import concourse.bass as bass
    import concourse.tile as tile
    import concourse.mybir as mybir
    from contextlib import ExitStack

    f16 = mybir.dt.float16
    f32 = mybir.dt.float32
    nc = bass.Bass()

    xs = nc.declare_dram_parameter("xs", [S, C, H, W], f16, isOutput=False)
    w2 = nc.declare_dram_parameter("w2", [128, 4, 3, MIP], f16, isOutput=False)
    bias2 = nc.declare_dram_parameter("bias2", [MIP, 1], f32, isOutput=False)
    gwv = nc.declare_dram_parameter("gwv", [MIP, 1], f32, isOutput=False)
    gbv = nc.declare_dram_parameter("gbv", [MIP, 1], f32, isOutput=False)
    wh = nc.declare_dram_parameter("wh", [MIP, C], f16, isOutput=False)
    ww = nc.declare_dram_parameter("ww", [MIP, C], f16, isOutput=False)
    bh = nc.declare_dram_parameter("bh", [128, 2], f32, isOutput=False)
    bw = nc.declare_dram_parameter("bw", [128, 2], f32, isOutput=False)
    out = nc.declare_dram_parameter("out", [S, C, H, W], f16, isOutput=True)

    with tile.TileContext(nc) as tc, ExitStack() as ctx:
        ctx.enter_context(nc.allow_low_precision(reason="2e-2 tolerance, fp16 path"))
        red = nc.vector
        Sig = mybir.ActivationFunctionType.Sigmoid

        singles = ctx.enter_context(tc.tile_pool(name="singles", bufs=1))
        xpool = ctx.enter_context(tc.tile_pool(name="xin", bufs=4))
        ypool = ctx.enter_context(tc.tile_pool(name="yall", bufs=2))
        small = ctx.enter_context(tc.tile_pool(name="small", bufs=4))
        tpool = ctx.enter_context(tc.tile_pool(name="tree", bufs=1))
        l3pool = ctx.enter_context(tc.tile_pool(name="l3", bufs=2))
        apool = ctx.enter_context(tc.tile_pool(name="attn", bufs=4))
        awpool = ctx.enter_context(tc.tile_pool(name="awp", bufs=4))
        pspool = ctx.enter_context(tc.tile_pool(name="ps", bufs=2, space="PSUM"))
        psgate = ctx.enter_context(tc.tile_pool(name="psg", bufs=3, space="PSUM"))

        # ---- x loads on the two HWDGE rings: sample 0 as ring-parallel
        # half-tiles (earliest possible pooling start), the rest as single
        # 2MB transfers alternating rings.  Params go on the idle GPSIMD
        # SWDGE queue so they never queue behind bulk x traffic. ----
        all_xts = []
        for s in range(S):
            xt = xpool.tile([128, 2 * HW], f16, tag="xt")
            all_xts.append(xt)
            for cb in range(2):
                src = xs[s, cb * 128:(cb + 1) * 128].rearrange("c h w -> c (h w)")
                dst = xt[:, cb * HW:(cb + 1) * HW]
                if s == 0 and cb == 0:
                    # quarter loads striped over both rings: first h-half of
                    # the first block lands earliest so pooling starts sooner
                    for q in range(4):
                        eng = nc.sync if q % 2 == 0 else nc.scalar
                        sl = slice(q * HW // 4, (q + 1) * HW // 4)
                        eng.dma_start(out=dst[:, sl], in_=src[:, sl])
                elif s == 0:
                    nc.sync.dma_start(out=dst[:, 0:HW // 2], in_=src[:, 0:HW // 2])
                    nc.scalar.dma_start(out=dst[:, HW // 2:HW], in_=src[:, HW // 2:HW])
                else:
                    eng = nc.sync if cb == 0 else nc.scalar
                    eng.dma_start(out=dst, in_=src)

        w2sb = singles.tile([128, 4, 3, MIP], f16)
        nc.gpsimd.dma_start(out=w2sb, in_=w2[:, :, :, :])
        bias2sb = singles.tile([MIP, 1], f32)
        nc.gpsimd.dma_start(out=bias2sb, in_=bias2[:, :])
        gwsb = singles.tile([MIP, 1], f32)
        nc.gpsimd.dma_start(out=gwsb, in_=gwv[:, :])
        gbsb = singles.tile([MIP, 1], f32)
        nc.gpsimd.dma_start(out=gbsb, in_=gbv[:, :])
        whsb = singles.tile([MIP, C], f16)
        nc.gpsimd.dma_start(out=whsb, in_=wh[:, :])
        wwsb = singles.tile([MIP, C], f16)
        nc.gpsimd.dma_start(out=wwsb, in_=ww[:, :])
        bhsb = singles.tile([128, 2], f32)
        nc.gpsimd.dma_start(out=bhsb, in_=bh[:, :])
        bwsb = singles.tile([128, 2], f32)
        nc.gpsimd.dma_start(out=bwsb, in_=bw[:, :])

        ahxs, aws = [], []
        for s in range(S):
            # ---------- pooling trees ----------
            # Sample 0 is processed per channel-block so pooling starts as
            # soon as the first half-tile lands; later samples use joint ops
            # over both blocks (halved instruction overhead).
            xt = all_xts[s]
            if s == 0:
                y_all = ypool.tile([128, 4, T], f16)
                yrow = y_all.rearrange("p (j c) t -> p j c t", j=2)
                ycol = y_all.rearrange("p (i c) t -> p i c t", i=2)
                # per-channel-block; block 0 additionally per h-half so the
                # first tree runs as soon as the first quarter-MB lands
                def half_trees(xc0, cb, i):
                    xh = xc0[:, i * (HW // 2):(i + 1) * (HW // 2)]
                    rv = xh.rearrange("p (h j w) -> p j h w", j=2, w=W // 2)
                    r1 = tpool.tile([128, 2, H // 2, 16], f16, tag="r1a")
                    red.tensor_add(out=r1, in0=rv[:, :, :, 0:16], in1=rv[:, :, :, 16:32])
                    r2 = tpool.tile([128, 2, H // 2, 8], f16, tag="r2a")
                    red.tensor_add(out=r2, in0=r1[:, :, :, 0:8], in1=r1[:, :, :, 8:16])
                    r3 = tpool.tile([128, 2, H // 2, 4], f16, tag="r3a")
                    red.tensor_add(out=r3, in0=r2[:, :, :, 0:4], in1=r2[:, :, :, 4:8])
                    r4 = tpool.tile([128, 2, H // 2, 2], f16, tag="r4a")
                    red.tensor_add(out=r4, in0=r3[:, :, :, 0:2], in1=r3[:, :, :, 2:4])
                    red.tensor_add(
                        out=yrow[:, :, cb, i * (H // 2):(i + 1) * (H // 2)],
                        in0=r4[:, :, :, 0], in1=r4[:, :, :, 1])
                    cv = xh.rearrange("p (m a w) -> p m a w", a=2, w=W)
                    c1 = tpool.tile([128, 16, W], f16, tag="c1a")
                    red.tensor_add(out=c1, in0=cv[:, :, 0], in1=cv[:, :, 1])
                    cv1 = c1.rearrange("p (m a) w -> p m a w", a=2)
                    c2 = tpool.tile([128, 8, W], f16, tag="c2a")
                    red.tensor_add(out=c2, in0=cv1[:, :, 0], in1=cv1[:, :, 1])
                    cv2 = c2.rearrange("p (m a) w -> p m a w", a=2)
                    c3 = tpool.tile([128, 4, W], f16, tag="c3a")
                    red.tensor_add(out=c3, in0=cv2[:, :, 0], in1=cv2[:, :, 1])
                    cv3 = c3.rearrange("p (m a) w -> p m a w", a=2)
                    c4 = tpool.tile([128, 2, W], f16, tag="c4a")
                    red.tensor_add(out=c4, in0=cv3[:, :, 0], in1=cv3[:, :, 1])
                    red.tensor_add(
                        out=ycol[:, i, cb, H:T],
                        in0=c4[:, 0, :], in1=c4[:, 1, :])

                for i in range(2):
                    half_trees(xt[:, 0:HW], 0, i)
                # block 1: full-block trees (its halves land while block 0
                # is being pooled)
                xc0 = xt[:, HW:2 * HW]
                rv = xc0.rearrange("p (h j w) -> p j h w", j=2, w=W // 2)
                r1 = tpool.tile([128, 2, H, 16], f16, tag="r1b")
                red.tensor_add(out=r1, in0=rv[:, :, :, 0:16], in1=rv[:, :, :, 16:32])
                r2 = tpool.tile([128, 2, H, 8], f16, tag="r2b")
                red.tensor_add(out=r2, in0=r1[:, :, :, 0:8], in1=r1[:, :, :, 8:16])
                r3 = tpool.tile([128, 2, H, 4], f16, tag="r3b")
                red.tensor_add(out=r3, in0=r2[:, :, :, 0:4], in1=r2[:, :, :, 4:8])
                r4 = tpool.tile([128, 2, H, 2], f16, tag="r4b")
                red.tensor_add(out=r4, in0=r3[:, :, :, 0:2], in1=r3[:, :, :, 2:4])
                red.tensor_add(
                    out=yrow[:, :, 1, 0:H],
                    in0=r4[:, :, :, 0], in1=r4[:, :, :, 1])
                cv = xc0.rearrange("p (i m a w) -> p i m a w", i=2, a=2, w=W)
                c1 = tpool.tile([128, 2, 16, W], f16, tag="c1b")
                red.tensor_add(out=c1, in0=cv[:, :, :, 0], in1=cv[:, :, :, 1])
                cv1 = c1.rearrange("p i (m a) w -> p i m a w", a=2)
                c2 = tpool.tile([128, 2, 8, W], f16, tag="c2b")
                red.tensor_add(out=c2, in0=cv1[:, :, :, 0], in1=cv1[:, :, :, 1])
                cv2 = c2.rearrange("p i (m a) w -> p i m a w", a=2)
                c3 = tpool.tile([128, 2, 4, W], f16, tag="c3b")
                red.tensor_add(out=c3, in0=cv2[:, :, :, 0], in1=cv2[:, :, :, 1])
                cv3 = c3.rearrange("p i (m a) w -> p i m a w", a=2)
                c4 = tpool.tile([128, 2, 2, W], f16, tag="c4b")
                red.tensor_add(out=c4, in0=cv3[:, :, :, 0], in1=cv3[:, :, :, 1])
                red.tensor_add(
                    out=ycol[:, :, 1, H:T],
                    in0=c4[:, :, 0, :], in1=c4[:, :, 1, :])
            else:
                # row-half sums: w-quarter tree stopped at L3 (the conv is
                # linear, so the PE consumes the four level-3 sub-sums per
                # group directly); (h, j) folds into one stride-32 dim
                rv = xt.rearrange("p (cb hj w) -> p cb hj w", cb=2, w=W // 2)
                r1 = tpool.tile([128, 2, 2 * H, 16], f16, tag="r1")
                red.tensor_add(out=r1, in0=rv[:, :, :, 0:16], in1=rv[:, :, :, 16:32])
                r2 = tpool.tile([128, 2, 2 * H, 8], f16, tag="r2")
                red.tensor_add(out=r2, in0=r1[:, :, :, 0:8], in1=r1[:, :, :, 8:16])
                r3 = l3pool.tile([128, 2, 2 * H, 4], f16, tag="r3")
                red.tensor_add(out=r3, in0=r2[:, :, :, 0:4], in1=r2[:, :, :, 4:8])
                # col-half sums: row-pair tree stopped at L3
                cv = xt.rearrange("p (ci m a w) -> p ci m a w", ci=4, a=2, w=W)
                c1 = tpool.tile([128, 4, 16, W], f16, tag="c1")
                red.tensor_add(out=c1, in0=cv[:, :, :, 0], in1=cv[:, :, :, 1])
                cv1 = c1.rearrange("p ci (m a) w -> p ci m a w", a=2)
                c2 = tpool.tile([128, 4, 8, W], f16, tag="c2")
                red.tensor_add(out=c2, in0=cv1[:, :, :, 0], in1=cv1[:, :, :, 1])
                cv2 = c2.rearrange("p ci (m a) w -> p ci m a w", a=2)
                c3 = l3pool.tile([128, 4, 4, W], f16, tag="c3")
                red.tensor_add(out=c3, in0=cv2[:, :, :, 0], in1=cv2[:, :, :, 1])

            # ---------- dilated conv on PE ----------
            psy = pspool.tile([MIP, T], f32, tag="psy")
            OO = {0: 2, 1: 0, 2: -2}
            IR = {0: (0, T - 2), 1: (0, T), 2: (2, T)}
            if s == 0:
                # from materialized y_all: 12 accumulated matmuls
                mms = []
                for g in range(4):
                    for k in range(3):
                        ilo, ihi = IR[k]
                        mms.append((w2sb[:, g, k, :], y_all[:, g, ilo:ihi],
                                    slice(ilo + OO[k], ihi + OO[k])))
            else:
                # from L3 sub-sums: 4 quarter-matmuls per (group, tap, region)
                r3v = r3.rearrange("p cb (h j) q -> p cb j h q", j=2)
                mms = []
                for g in range(4):
                    ji, cb = g >> 1, g & 1
                    for k in range(3):
                        ilo, ihi = IR[k]
                        a, b = ilo, min(ihi, H)
                        for q in range(4):
                            mms.append((w2sb[:, g, k, :], r3v[:, cb, ji, a:b, q],
                                        slice(a + OO[k], b + OO[k])))
                        a2, b2 = max(ilo, H), ihi
                        ci = cb * 2 + ji
                        for q in range(4):
                            mms.append((w2sb[:, g, k, :],
                                        c3[:, ci, q, a2 - H:b2 - H],
                                        slice(a2 + OO[k], b2 + OO[k])))
            for idx, (lhsT, rhs, o_sl) in enumerate(mms):
                nc.tensor.matmul(
                    out=psy[:, o_sl], lhsT=lhsT, rhs=rhs,
                    start=(idx == 0), stop=(idx == len(mms) - 1),
                )

            # ---------- bias + BN (folded) + SiLU + SE (sigmoid-only ACT) ----
            ya0 = small.tile([MIP, T], f32, tag="ya0")
            nc.vector.tensor_scalar_add(out=ya0, in0=psy, scalar1=bias2sb[:, :])
            ysg = small.tile([MIP, T], f32, tag="ysg")
            nc.scalar.activation(out=ysg, in_=ya0, func=Sig, bias=0.0, scale=1.0)
            ya = small.tile([MIP, T], f32, tag="ya")
            red.tensor_mul(out=ya, in0=ya0, in1=ysg)
            ysum = small.tile([MIP, 1], f32, tag="ysum")
            red.reduce_sum(out=ysum, in_=ya, axis=mybir.AxisListType.X)
            se = small.tile([MIP, 1], f32, tag="se")
            nc.scalar.activation(out=se, in_=ysum, func=Sig,
                                 bias=gbsb[:, :], scale=gwsb[:, :])
            yg = small.tile([MIP, T], f16, tag="yg")
            nc.vector.tensor_scalar_mul(out=yg, in0=ya, scalar1=se[:, :])

            # ---------- h/w attention gates (PE + ACT only; muls deferred) ---
            ahx2 = apool.tile([128, 2, H, W], f16, tag="ahx")
            aw2 = awpool.tile([128, 2, W], f16, tag="aw")
            ahxs.append(ahx2)
            aws.append(aw2)
            for cb in range(2):
                psa = psgate.tile([128, H], f32, tag="psa")
                nc.tensor.matmul(
                    out=psa,
                    lhsT=whsb[:, cb * 128:(cb + 1) * 128],
                    rhs=yg[:, 0:H], start=True, stop=True,
                )
                # fused sigmoid + broadcast-expand on ACT: ahx[p,h,w]=sig(psa[p,h]+bh)
                pa = psa[:, :]
                pab = bass.AP(tensor=pa.tensor, offset=pa.offset,
                              ap=[pa.ap[0], pa.ap[1], [0, W]])
                nc.scalar.activation(out=ahx2[:, cb], in_=pab, func=Sig,
                                     bias=bhsb[:, cb:cb + 1], scale=1.0)
                psb = psgate.tile([128, W], f32, tag="psb")
                nc.tensor.matmul(
                    out=psb,
                    lhsT=wwsb[:, cb * 128:(cb + 1) * 128],
                    rhs=yg[:, H:T], start=True, stop=True,
                )
                nc.scalar.activation(out=aw2[:, cb], in_=psb, func=Sig,
                                     bias=bwsb[:, cb:cb + 1], scale=1.0)

        # ---------- final multiplies (both 2x on DVE, in place) + stores -----
        for s in range(S):
            xt = all_xts[s]
            xv = xt.rearrange("p (ch w) -> p ch w", w=W)
            av = ahxs[s].rearrange("p cb h w -> p (cb h) w")
            red.tensor_mul(out=xv, in0=xv, in1=av)
            a = aws[s][:, :, :]
            awb = bass.AP(tensor=a.tensor, offset=a.offset,
                          ap=[a.ap[0], a.ap[1], [0, H], a.ap[2]])
            xc = xt.rearrange("p (cb h w) -> p cb h w", cb=2, w=W)
            ost = out[s].rearrange("(cb c) h w -> c cb (h w)", cb=2)
            if s < S - 1:
                red.tensor_mul(out=xc, in0=xc, in1=awb)
                # store halves on both rings so no single ring backs up
                nc.sync.dma_start(out=ost[:, 0], in_=xt[:, 0:HW])
                nc.scalar.dma_start(out=ost[:, 1], in_=xt[:, HW:2 * HW])
            else:
                # last sample: mul2 in h-half chunks with eighth-tile stores
                # striped over both rings so the final transfer is tiny
                for cb in range(2):
                    acb = aws[s][:, cb, :]
                    for hh in range(2):
                        hsl = slice(hh * (H // 2), (hh + 1) * (H // 2))
                        red.tensor_mul(
                            out=xc[:, cb, hsl], in0=xc[:, cb, hsl],
                            in1=bass.AP(tensor=acb.tensor, offset=acb.offset,
                                        ap=[acb.ap[0], [0, H // 2], acb.ap[1]]))
                        for qq in range(2):
                            eng = nc.sync if qq == 0 else nc.scalar
                            lo = cb * HW + hh * (HW // 2) + qq * (HW // 4)
                            osl = slice(hh * (HW // 2) + qq * (HW // 4),
                                        hh * (HW // 2) + (qq + 1) * (HW // 4))
                            eng.dma_start(out=ost[:, cb, osl],
                                          in_=xt[:, lo:lo + HW // 4])
    if legalize:
        _legalize_waits(nc, mybir)
    return nc


def _prep_params(conv1_w, conv1_b, bn_gamma, bn_beta, bn_mean, bn_var,
                 gate_w, gate_b, convh_w, convh_b, convw_w, convw_b):
    f32 = np.float32
    bnscale = (np.asarray(bn_gamma, f32)
               / np.sqrt(np.asarray(bn_var, f32) + BN_EPS)).astype(f32)
    Wc = np.asarray(conv1_w, f32)[:, :, :, 1]                     # [MIP, 768, 3]
    s_ci = np.where(np.arange(3 * C) < C, 1.0 / W, 2.0 / W).astype(f32)
    W2 = (Wc * s_ci[None, :, None] * bnscale[:, None, None]).astype(f32)
    bias2 = ((np.asarray(conv1_b, f32) - np.asarray(bn_mean, f32)) * bnscale
             + np.asarray(bn_beta, f32)).astype(f32)
    # Fold the full-mean channel blocks (g=0,1) into the four half-sum
    # blocks: conv is linear and fullsum = half0sum + half1sum, so
    # W'[g'] = W[2+g'] + W[g'%2] and only 4 channel-blocks remain.
    W6 = W2.reshape(MIP, 6, 128, 3)
    W4 = np.stack([W6[:, 2 + gp] + W6[:, gp % 2] for gp in range(4)], axis=1)
    # w2 layout [ci_local=128, g'=4, k=3, o=MIP]
    w2 = np.ascontiguousarray(W4.transpose(2, 1, 3, 0)).astype(np.float16)
    gw = np.full((MIP, 1), float(gate_w) / T, f32)
    gb = np.full((MIP, 1), float(gate_b), f32)
    wh = np.ascontiguousarray(np.asarray(convh_w, np.float16).T)   # [MIP, 256]
    ww = np.ascontiguousarray(np.asarray(convw_w, np.float16).T)
    bh = np.ascontiguousarray(np.asarray(convh_b, f32).reshape(2, 128).T)  # [128, 2]
    bw = np.ascontiguousarray(np.asarray(convw_b, f32).reshape(2, 128).T)
    return dict(w2=w2, bias2=bias2.reshape(MIP, 1), gwv=gw, gbv=gb,
                wh=wh, ww=ww, bh=bh, bw=bw)


def kernel(**inputs):
    import sys
    if "/opt/trn_rl_repo" not in sys.path:
        sys.path.insert(0, "/opt/trn_rl_repo")
    from concourse.bass_utils import run_bass_kernel_spmd

    x = np.asarray(inputs["x"], np.float32).astype(np.float16)
    params = _prep_params(
        inputs["conv1_w"], inputs["conv1_b"], inputs["bn_gamma"],
        inputs["bn_beta"], inputs["bn_mean"], inputs["bn_var"],
        inputs["gate_w"], inputs["gate_b"], inputs["convh_w"],
        inputs["convh_b"], inputs["convw_w"], inputs["convw_b"])

    if "nc" not in _CACHE:
        _CACHE["nc"] = _build_program()
    nc = _CACHE["nc"]

    in_maps = [
        {"xs": np.ascontiguousarray(x[i * S:(i + 1) * S]), **params}
        for i in range(N_CORES)
    ]
    res = run_bass_kernel_spmd(nc, in_maps, core_ids=list(range(N_CORES)))
    out = np.concatenate([r["out"] for r in res.results], axis=0)
    return out.astype(np.float32)


# revision 23
# speedup vs baseline: 1.1274x; 1.1274x over previous
"""Trainium2 Bass kernel for nn_EnhancedCoordAtt (coordinate attention).

Strategy (pure data parallel, 8 cores x 4 samples, fp16 data path):
  - x [32,256,64,64] is converted to fp16 on the host (tolerance is 2e-2;
    fp16 keeps us ~1e-3) and sharded on batch; per core, per (sample,
    channel-block of 128) a [128, 4096] fp16 SBUF tile is streamed in.
    fp16 halves HBM traffic and doubles DVE tensor_tensor throughput
    (2x_1p packed mode; DVE reduce has no fast mode, so reduces are
    replaced by pairwise TT add trees which do run at 2x).
  - Pooling: row-half sums via a w-quarter add tree (4 levels + tiny pair
    op), column-half sums via a row-pair add tree (5 levels), both writing
    into a [128, 4, 128] y-tensor (concat([xh, xw]) layout with the
    full-mean channels folded into the weights host-side).
  - The dilated 3x3 conv is 12 accumulated PE matmuls into [8,128] PSUM;
    SiLU is done sigmoid-only (ACT sigmoid + fused DVE mul+reduce) so the
    ACT engine never swaps activation tables; SE gate on ACT.
  - Gate application: the h-gate sigmoid is fused with a broadcast
    expansion on the (otherwise idle) ACT engine into a full [128,64,64]
    fp16 tile so that both final multiplies have unit-stride operands and
    hit the DVE 2x mode; the w-gate broadcasts along a middle dim which
    is 2x-eligible directly.  out = (x * ahx) * aw_bcast, stored fp16 and
    upconverted to f32 on the host.
All shapes/constants hardcoded to the nn_EnhancedCoordAtt_78855599555233 spec.
"""

import numpy as np

N, C, H, W = 32, 256, 64, 64
MIP = 8
N_CORES = 8
S = N // N_CORES           # samples per core
HW = H * W
T = H + W                  # 128
BN_EPS = 1e-5

_CACHE = {}


def _legalize_waits(nc, mybir, max_keep=1):
    """walrus encodes at most one sync-wait on most compute/DMA ISA structs.
    Move excess waits onto standalone EventSemaphore (wait-only) instructions
    inserted immediately before the offender in its engine stream."""
    f = nc.m.functions[0]
    for blk in f.blocks:
        out, changed = [], False
        for inst in blk.instructions:
            si = inst.sync_info
            t = type(inst).__name__
            if (si is not None and len(si.on_wait) > max_keep
                    and t != "InstEventSemaphore"):
                waits = list(si.on_wait)
                for j, w in enumerate(waits[:-max_keep]):
                    ev = mybir.InstEventSemaphore(
                        name=f"{inst.name}_xw{j}", ins=[], outs=[])
                    ev.engine = inst.engine
                    ev.sync_info = mybir.SyncInfo(on_wait=[w], on_update=[])
                    out.append(ev)
                inst.sync_info = mybir.SyncInfo(
                    on_wait=waits[-max_keep:], on_update=list(si.on_update))
                changed = True
            out.append(inst)
        if changed:
            blk.instructions = out


def _build_program(legalize=True, sim_compat=False):
    import concourse.bass as bass
    import concourse.tile as tile
    import concourse.mybir as mybir
    from contextlib import ExitStack

    f16 = mybir.dt.float16
    f32 = mybir.dt.float32
    nc = bass.Bass()

    xs = nc.declare_dram_parameter("xs", [S, C, H, W], f16, isOutput=False)
    w2 = nc.declare_dram_parameter("w2", [128, 4, 3, MIP], f16, isOutput=False)
    bias2 = nc.declare_dram_parameter("bias2", [MIP, 1], f32, isOutput=False)
    gwv = nc.declare_dram_parameter("gwv", [MIP, 1], f32, isOutput=False)
    gbv = nc.declare_dram_parameter("gbv", [MIP, 1], f32, isOutput=False)
    wh = nc.declare_dram_parameter("wh", [MIP, C], f16, isOutput=False)
    ww = nc.declare_dram_parameter("ww", [MIP, C], f16, isOutput=False)
    bh = nc.declare_dram_parameter("bh", [128, 2], f32, isOutput=False)
    bw = nc.declare_dram_parameter("bw", [128, 2], f32, isOutput=False)
    out = nc.declare_dram_parameter("out", [S, C, H, W], f16, isOutput=True)

    with tile.TileContext(nc) as tc, ExitStack() as ctx:
        ctx.enter_context(nc.allow_low_precision(reason="2e-2 tolerance, fp16 path"))
        red = nc.vector
        Sig = mybir.ActivationFunctionType.Sigmoid

        singles = ctx.enter_context(tc.tile_pool(name="singles", bufs=1))
        xpool = ctx.enter_context(tc.tile_pool(name="xin", bufs=4))
        ypool = ctx.enter_context(tc.tile_pool(name="yall", bufs=2))
        small = ctx.enter_context(tc.tile_pool(name="small", bufs=4))
        tpool = ctx.enter_context(tc.tile_pool(name="tree", bufs=1))
        l3pool = ctx.enter_context(tc.tile_pool(name="l3", bufs=2))
        apool = ctx.enter_context(tc.tile_pool(name="attn", bufs=4))
        awpool = ctx.enter_context(tc.tile_pool(name="awp", bufs=4))
        pspool = ctx.enter_context(tc.tile_pool(name="ps", bufs=2, space="PSUM"))
        psgate = ctx.enter_context(tc.tile_pool(name="psg", bufs=3, space="PSUM"))

        # ---- x loads on the two HWDGE rings: sample 0 as ring-parallel
        # half-tiles (earliest possible pooling start), the rest as single
        # 2MB transfers alternating rings.  Params go on the idle GPSIMD
        # SWDGE queue so they never queue behind bulk x traffic. ----
        all_xts = []
        for s in range(S):
            xt = xpool.tile([128, 2 * HW], f16, tag="xt")
            all_xts.append(xt)
            for cb in range(2):
                src = xs[s, cb * 128:(cb + 1) * 128].rearrange("c h w -> c (h w)")
                dst = xt[:, cb * HW:(cb + 1) * HW]
                if s == 0 and cb == 0:
                    # quarter loads striped over both rings: first h-half of
                    # the first block lands earliest so pooling starts sooner
                    for q in range(4):
                        eng = nc.sync if q % 2 == 0 else nc.scalar
                        sl = slice(q * HW // 4, (q + 1) * HW // 4)
                        eng.dma_start(out=dst[:, sl], in_=src[:, sl])
                elif s == 0:
                    nc.sync.dma_start(out=dst[:, 0:HW // 2], in_=src[:, 0:HW // 2])
                    nc.scalar.dma_start(out=dst[:, HW // 2:HW], in_=src[:, HW // 2:HW])
                else:
                    eng = nc.sync if cb == 0 else nc.scalar
                    eng.dma_start(out=dst, in_=src)

        w2sb = singles.tile([128, 4, 3, MIP], f16)
        nc.gpsimd.dma_start(out=w2sb, in_=w2[:, :, :, :])
        bias2sb = singles.tile([MIP, 1], f32)
        nc.gpsimd.dma_start(out=bias2sb, in_=bias2[:, :])
        gwsb = singles.tile([MIP, 1], f32)
        nc.gpsimd.dma_start(out=gwsb, in_=gwv[:, :])
        gbsb = singles.tile([MIP, 1], f32)
        nc.gpsimd.dma_start(out=gbsb, in_=gbv[:, :])
        whsb = singles.tile([MIP, C], f16)
        nc.gpsimd.dma_start(out=whsb, in_=wh[:, :])
        wwsb = singles.tile([MIP, C], f16)
        nc.gpsimd.dma_start(out=wwsb, in_=ww[:, :])
        bhsb = singles.tile([128, 2], f32)
        nc.gpsimd.dma_start(out=bhsb, in_=bh[:, :])
        bwsb = singles.tile([128, 2], f32)
        nc.gpsimd.dma_start(out=bwsb, in_=bw[:, :])

        ahxs, aws = [], []
        for s in range(S):
            # ---------- pooling trees ----------
            # Sample 0 is processed per channel-block so pooling starts as
            # soon as the first half-tile lands; later samples use joint ops
            # over both blocks (halved instruction overhead).
            xt = all_xts[s]
            if s == 0:
                y_all = ypool.tile([128, 4, T], f16)
                yrow = y_all.rearrange("p (j c) t -> p j c t", j=2)
                ycol = y_all.rearrange("p (i c) t -> p i c t", i=2)
                # per-channel-block; block 0 additionally per h-half so the
                # first tree runs as soon as the first quarter-MB lands
                def half_trees(xc0, cb, i):
                    xh = xc0[:, i * (HW // 2):(i + 1) * (HW // 2)]
                    rv = xh.rearrange("p (h j w) -> p j h w", j=2, w=W // 2)
                    r1 = tpool.tile([128, 2, H // 2, 16], f16, tag="r1a")
                    red.tensor_add(out=r1, in0=rv[:, :, :, 0:16], in1=rv[:, :, :, 16:32])
                    r2 = tpool.tile([128, 2, H // 2, 8], f16, tag="r2a")
                    red.tensor_add(out=r2, in0=r1[:, :, :, 0:8], in1=r1[:, :, :, 8:16])
                    r3 = tpool.tile([128, 2, H // 2, 4], f16, tag="r3a")
                    red.tensor_add(out=r3, in0=r2[:, :, :, 0:4], in1=r2[:, :, :, 4:8])
                    r4 = tpool.tile([128, 2, H // 2, 2], f16, tag="r4a")
                    red.tensor_add(out=r4, in0=r3[:, :, :, 0:2], in1=r3[:, :, :, 2:4])
                    red.tensor_add(
                        out=yrow[:, :, cb, i * (H // 2):(i + 1) * (H // 2)],
                        in0=r4[:, :, :, 0], in1=r4[:, :, :, 1])
                    cv = xh.rearrange("p (m a w) -> p m a w", a=2, w=W)
                    c1 = tpool.tile([128, 16, W], f16, tag="c1a")
                    red.tensor_add(out=c1, in0=cv[:, :, 0], in1=cv[:, :, 1])
                    cv1 = c1.rearrange("p (m a) w -> p m a w", a=2)
                    c2 = tpool.tile([128, 8, W], f16, tag="c2a")
                    red.tensor_add(out=c2, in0=cv1[:, :, 0], in1=cv1[:, :, 1])
                    cv2 = c2.rearrange("p (m a) w -> p m a w", a=2)
                    c3 = tpool.tile([128, 4, W], f16, tag="c3a")
                    red.tensor_add(out=c3, in0=cv2[:, :, 0], in1=cv2[:, :, 1])
                    cv3 = c3.rearrange("p (m a) w -> p m a w", a=2)
                    c4 = tpool.tile([128, 2, W], f16, tag="c4a")
                    red.tensor_add(out=c4, in0=cv3[:, :, 0], in1=cv3[:, :, 1])
                    red.tensor_add(
                        out=ycol[:, i, cb, H:T],
                        in0=c4[:, 0, :], in1=c4[:, 1, :])

                for i in range(2):
                    half_trees(xt[:, 0:HW], 0, i)
                # block 1: full-block trees (its halves land while block 0
                # is being pooled)
                xc0 = xt[:, HW:2 * HW]
                rv = xc0.rearrange("p (h j w) -> p j h w", j=2, w=W // 2)
                r1 = tpool.tile([128, 2, H, 16], f16, tag="r1b")
                red.tensor_add(out=r1, in0=rv[:, :, :, 0:16], in1=rv[:, :, :, 16:32])
                r2 = tpool.tile([128, 2, H, 8], f16, tag="r2b")
                red.tensor_add(out=r2, in0=r1[:, :, :, 0:8], in1=r1[:, :, :, 8:16])
                r3 = tpool.tile([128, 2, H, 4], f16, tag="r3b")
                red.tensor_add(out=r3, in0=r2[:, :, :, 0:4], in1=r2[:, :, :, 4:8])
                r4 = tpool.tile([128, 2, H, 2], f16, tag="r4b")
                red.tensor_add(out=r4, in0=r3[:, :, :, 0:2], in1=r3[:, :, :, 2:4])
                red.tensor_add(
                    out=yrow[:, :, 1, 0:H],
                    in0=r4[:, :, :, 0], in1=r4[:, :, :, 1])
                cv = xc0.rearrange("p (i m a w) -> p i m a w", i=2, a=2, w=W)
                c1 = tpool.tile([128, 2, 16, W], f16, tag="c1b")
                red.tensor_add(out=c1, in0=cv[:, :, :, 0], in1=cv[:, :, :, 1])
                cv1 = c1.rearrange("p i (m a) w -> p i m a w", a=2)
                c2 = tpool.tile([128, 2, 8, W], f16, tag="c2b")
                red.tensor_add(out=c2, in0=cv1[:, :, :, 0], in1=cv1[:, :, :, 1])
                cv2 = c2.rearrange("p i (m a) w -> p i m a w", a=2)
                c3 = tpool.tile([128, 2, 4, W], f16, tag="c3b")
                red.tensor_add(out=c3, in0=cv2[:, :, :, 0], in1=cv2[:, :, :, 1])
                cv3 = c3.rearrange("p i (m a) w -> p i m a w", a=2)
                c4 = tpool.tile([128, 2, 2, W], f16, tag="c4b")
                red.tensor_add(out=c4, in0=cv3[:, :, :, 0], in1=cv3[:, :, :, 1])
                red.tensor_add(
                    out=ycol[:, :, 1, H:T],
                    in0=c4[:, :, 0, :], in1=c4[:, :, 1, :])
            else:
                y_all = ypool.tile([128, 4, T], f16)
                # row-half sums: w-quarter tree; (h, j) folds into one
                # stride-32 dim so joint ops stay within 3 free dims
                rv = xt.rearrange("p (cb hj w) -> p cb hj w", cb=2, w=W // 2)
                r1 = tpool.tile([128, 2, 2 * H, 16], f16, tag="r1")
                red.tensor_add(out=r1, in0=rv[:, :, :, 0:16], in1=rv[:, :, :, 16:32])
                r2 = tpool.tile([128, 2, 2 * H, 8], f16, tag="r2")
                red.tensor_add(out=r2, in0=r1[:, :, :, 0:8], in1=r1[:, :, :, 8:16])
                r3 = tpool.tile([128, 2, 2 * H, 4], f16, tag="r3")
                red.tensor_add(out=r3, in0=r2[:, :, :, 0:4], in1=r2[:, :, :, 4:8])
                r4 = tpool.tile([128, 2, 2 * H, 2], f16, tag="r4")
                red.tensor_add(out=r4, in0=r3[:, :, :, 0:2], in1=r3[:, :, :, 2:4])
                rv4 = r4.rearrange("p cb (h j) a -> p cb j h a", j=2)
                red.tensor_add(
                    out=y_all.rearrange("p (j c) t -> p c j t", j=2)[:, :, :, 0:H],
                    in0=rv4[:, :, :, :, 0], in1=rv4[:, :, :, :, 1])
                # col-half sums: row-pair tree ((cb, i) folds to stride 2048)
                cv = xt.rearrange("p (ci m a w) -> p ci m a w", ci=4, a=2, w=W)
                c1 = tpool.tile([128, 4, 16, W], f16, tag="c1")
                red.tensor_add(out=c1, in0=cv[:, :, :, 0], in1=cv[:, :, :, 1])
                cv1 = c1.rearrange("p ci (m a) w -> p ci m a w", a=2)
                c2 = tpool.tile([128, 4, 8, W], f16, tag="c2")
                red.tensor_add(out=c2, in0=cv1[:, :, :, 0], in1=cv1[:, :, :, 1])
                cv2 = c2.rearrange("p ci (m a) w -> p ci m a w", a=2)
                c3 = tpool.tile([128, 4, 4, W], f16, tag="c3")
                red.tensor_add(out=c3, in0=cv2[:, :, :, 0], in1=cv2[:, :, :, 1])
                cv3 = c3.rearrange("p ci (m a) w -> p ci m a w", a=2)
                c4 = tpool.tile([128, 4, 2, W], f16, tag="c4")
                red.tensor_add(out=c4, in0=cv3[:, :, :, 0], in1=cv3[:, :, :, 1])
                cv4 = c4.rearrange("p (cb i) a w -> p cb i a w", cb=2)
                red.tensor_add(
                    out=y_all.rearrange("p (i c) t -> p c i t", i=2)[:, :, :, H:T],
                    in0=cv4[:, :, :, 0, :], in1=cv4[:, :, :, 1, :])

            # ---------- dilated conv as 12 accumulated matmuls ----------
            psy = pspool.tile([MIP, T], f32, tag="psy")
            order = [(0, 1)] + [(g, k) for g in range(4) for k in range(3) if (g, k) != (0, 1)]
            for idx, (g, k) in enumerate(order):
                lhsT = w2sb[:, g, k, :]
                if k == 1:
                    o_sl, i_sl = slice(0, T), slice(0, T)
                elif k == 0:
                    o_sl, i_sl = slice(2, T), slice(0, T - 2)
                else:
                    o_sl, i_sl = slice(0, T - 2), slice(2, T)
                nc.tensor.matmul(
                    out=psy[:, o_sl],
                    lhsT=lhsT,
                    rhs=y_all[:, g, i_sl],
                    start=(idx == 0),
                    stop=(idx == len(order) - 1),
                )

            # ---------- bias + BN (folded) + SiLU + SE (sigmoid-only ACT) ----
            ya0 = small.tile([MIP, T], f32, tag="ya0")
            nc.vector.tensor_scalar_add(out=ya0, in0=psy, scalar1=bias2sb[:, :])
            ysg = small.tile([MIP, T], f32, tag="ysg")
            nc.scalar.activation(out=ysg, in_=ya0, func=Sig, bias=0.0, scale=1.0)
            ya = small.tile([MIP, T], f32, tag="ya")
            red.tensor_mul(out=ya, in0=ya0, in1=ysg)
            ysum = small.tile([MIP, 1], f32, tag="ysum")
            red.reduce_sum(out=ysum, in_=ya, axis=mybir.AxisListType.X)
            se = small.tile([MIP, 1], f32, tag="se")
            nc.scalar.activation(out=se, in_=ysum, func=Sig,
                                 bias=gbsb[:, :], scale=gwsb[:, :])
            yg = small.tile([MIP, T], f16, tag="yg")
            nc.vector.tensor_scalar_mul(out=yg, in0=ya, scalar1=se[:, :])

            # ---------- h/w attention gates (PE + ACT only; muls deferred) ---
            ahx2 = apool.tile([128, 2, H, W], f16, tag="ahx")
            aw2 = awpool.tile([128, 2, W], f16, tag="aw")
            ahxs.append(ahx2)
            aws.append(aw2)
            for cb in range(2):
                psa = psgate.tile([128, H], f32, tag="psa")
                nc.tensor.matmul(
                    out=psa,
                    lhsT=whsb[:, cb * 128:(cb + 1) * 128],
                    rhs=yg[:, 0:H], start=True, stop=True,
                )
                # fused sigmoid + broadcast-expand on ACT: ahx[p,h,w]=sig(psa[p,h]+bh)
                pa = psa[:, :]
                pab = bass.AP(tensor=pa.tensor, offset=pa.offset,
                              ap=[pa.ap[0], pa.ap[1], [0, W]])
                nc.scalar.activation(out=ahx2[:, cb], in_=pab, func=Sig,
                                     bias=bhsb[:, cb:cb + 1], scale=1.0)
                psb = psgate.tile([128, W], f32, tag="psb")
                nc.tensor.matmul(
                    out=psb,
                    lhsT=wwsb[:, cb * 128:(cb + 1) * 128],
                    rhs=yg[:, H:T], start=True, stop=True,
                )
                nc.scalar.activation(out=aw2[:, cb], in_=psb, func=Sig,
                                     bias=bwsb[:, cb:cb + 1], scale=1.0)

        # ---------- final multiplies (both 2x on DVE, in place) + stores -----
        for s in range(S):
            xt = all_xts[s]
            xv = xt.rearrange("p (ch w) -> p ch w", w=W)
            av = ahxs[s].rearrange("p cb h w -> p (cb h) w")
            red.tensor_mul(out=xv, in0=xv, in1=av)
            a = aws[s][:, :, :]
            awb = bass.AP(tensor=a.tensor, offset=a.offset,
                          ap=[a.ap[0], a.ap[1], [0, H], a.ap[2]])
            xc = xt.rearrange("p (cb h w) -> p cb h w", cb=2, w=W)
            ost = out[s].rearrange("(cb c) h w -> c cb (h w)", cb=2)
            if s < S - 1:
                red.tensor_mul(out=xc, in0=xc, in1=awb)
                # store halves on both rings so no single ring backs up
                nc.sync.dma_start(out=ost[:, 0], in_=xt[:, 0:HW])
                nc.scalar.dma_start(out=ost[:, 1], in_=xt[:, HW:2 * HW])
            else:
                # last sample: mul2 in h-half chunks with eighth-tile stores
                # striped over both rings so the final transfer is tiny
                for cb in range(2):
                    acb = aws[s][:, cb, :]
                    for hh in range(2):
                        hsl = slice(hh * (H // 2), (hh + 1) * (H // 2))
                        red.tensor_mul(
                            out=xc[:, cb, hsl], in0=xc[:, cb, hsl],
                            in1=bass.AP(tensor=acb.tensor, offset=acb.offset,
                                        ap=[acb.ap[0], [0, H // 2], acb.ap[1]]))
                        for qq in range(2):
                            eng = nc.sync if qq == 0 else nc.scalar
                            lo = cb * HW + hh * (HW // 2) + qq * (HW // 4)
                            osl = slice(hh * (HW // 2) + qq * (HW // 4),
                                        hh * (HW // 2) + (qq + 1) * (HW // 4))
                            eng.dma_start(out=ost[:, cb, osl],
                                          in_=xt[:, lo:lo + HW // 4])
    if legalize:
        _legalize_waits(nc, mybir)
    return nc


def _prep_params(conv1_w, conv1_b, bn_gamma, bn_beta, bn_mean, bn_var,
                 gate_w, gate_b, convh_w, convh_b, convw_w, convw_b):
    f32 = np.float32
    bnscale = (np.asarray(bn_gamma, f32)
               / np.sqrt(np.asarray(bn_var, f32) + BN_EPS)).astype(f32)
    Wc = np.asarray(conv1_w, f32)[:, :, :, 1]                     # [MIP, 768, 3]
    s_ci = np.where(np.arange(3 * C) < C, 1.0 / W, 2.0 / W).astype(f32)
    W2 = (Wc * s_ci[None, :, None] * bnscale[:, None, None]).astype(f32)
    bias2 = ((np.asarray(conv1_b, f32) - np.asarray(bn_mean, f32)) * bnscale
             + np.asarray(bn_beta, f32)).astype(f32)
    # Fold the full-mean channel blocks (g=0,1) into the four half-sum
    # blocks: conv is linear and fullsum = half0sum + half1sum, so
    # W'[g'] = W[2+g'] + W[g'%2] and only 4 channel-blocks remain.
    W6 = W2.reshape(MIP, 6, 128, 3)
    W4 = np.stack([W6[:, 2 + gp] + W6[:, gp % 2] for gp in range(4)], axis=1)
    # w2 layout [ci_local=128, g'=4, k=3, o=MIP]
    w2 = np.ascontiguousarray(W4.transpose(2, 1, 3, 0)).astype(np.float16)
    gw = np.full((MIP, 1), float(gate_w) / T, f32)
    gb = np.full((MIP, 1), float(gate_b), f32)
    wh = np.ascontiguousarray(np.asarray(convh_w, np.float16).T)   # [MIP, 256]
    ww = np.ascontiguousarray(np.asarray(convw_w, np.float16).T)
    bh = np.ascontiguousarray(np.asarray(convh_b, f32).reshape(2, 128).T)  # [128, 2]
    bw = np.ascontiguousarray(np.asarray(convw_b, f32).reshape(2, 128).T)
    return dict(w2=w2, bias2=bias2.reshape(MIP, 1), gwv=gw, gbv=gb,
                wh=wh, ww=ww, bh=bh, bw=bw)


def kernel(**inputs):
    import sys
    if "/opt/trn_rl_repo" not in sys.path:
        sys.path.insert(0, "/opt/trn_rl_repo")
    from concourse.bass_utils import run_bass_kernel_spmd

    x = np.asarray(inputs["x"], np.float32).astype(np.float16)
    params = _prep_params(
        inputs["conv1_w"], inputs["conv1_b"], inputs["bn_gamma"],
        inputs["bn_beta"], inputs["bn_mean"], inputs["bn_var"],
        inputs["gate_w"], inputs["gate_b"], inputs["convh_w"],
        inputs["convh_b"], inputs["convw_w"], inputs["convw_b"])

    if "nc" not in _CACHE:
        _CACHE["nc"] = _build_program()
    nc = _CACHE["nc"]

    in_maps = [
        {"xs": np.ascontiguousarray(x[i * S:(i + 1) * S]), **params}
        for i in range(N_CORES)
    ]
    res = run_bass_kernel_spmd(nc, in_maps, core_ids=list(range(N_CORES)))
    out = np.concatenate([r["out"] for r in res.results], axis=0)
    return out.astype(np.float32)


# revision 24
# speedup vs baseline: 1.1531x; 1.0228x over previous
"""Trainium2 Bass kernel for nn_EnhancedCoordAtt (coordinate attention).

Strategy (pure data parallel, 8 cores x 4 samples, fp16 data path):
  - x [32,256,64,64] is converted to fp16 on the host (tolerance is 2e-2;
    fp16 keeps us ~1e-3) and sharded on batch; per core, per (sample,
    channel-block of 128) a [128, 4096] fp16 SBUF tile is streamed in.
    fp16 halves HBM traffic and doubles DVE tensor_tensor throughput
    (2x_1p packed mode; DVE reduce has no fast mode, so reduces are
    replaced by pairwise TT add trees which do run at 2x).
  - Pooling: row-half sums via a w-quarter add tree (4 levels + tiny pair
    op), column-half sums via a row-pair add tree (5 levels), both writing
    into a [128, 4, 128] y-tensor (concat([xh, xw]) layout with the
    full-mean channels folded into the weights host-side).
  - The dilated 3x3 conv is 12 accumulated PE matmuls into [8,128] PSUM;
    SiLU is done sigmoid-only (ACT sigmoid + fused DVE mul+reduce) so the
    ACT engine never swaps activation tables; SE gate on ACT.
  - Gate application: the h-gate sigmoid is fused with a broadcast
    expansion on the (otherwise idle) ACT engine into a full [128,64,64]
    fp16 tile so that both final multiplies have unit-stride operands and
    hit the DVE 2x mode; the w-gate broadcasts along a middle dim which
    is 2x-eligible directly.  out = (x * ahx) * aw_bcast, stored fp16 and
    upconverted to f32 on the host.
All shapes/constants hardcoded to the nn_EnhancedCoordAtt_78855599555233 spec.
"""

import numpy as np

N, C, H, W = 32, 256, 64, 64
MIP = 8
N_CORES = 8
S = N // N_CORES           # samples per core
HW = H * W
T = H + W                  # 128
BN_EPS = 1e-5

_CACHE = {}


def _legalize_waits(nc, mybir, max_keep=1):
    """walrus encodes at most one sync-wait on most compute/DMA ISA structs.
    Move excess waits onto standalone EventSemaphore (wait-only) instructions
    inserted immediately before the offender in its engine stream."""
    f = nc.m.functions[0]
    for blk in f.blocks:
        out, changed = [], False
        for inst in blk.instructions:
            si = inst.sync_info
            t = type(inst).__name__
            if (si is not None and len(si.on_wait) > max_keep
                    and t != "InstEventSemaphore"):
                waits = list(si.on_wait)
                for j, w in enumerate(waits[:-max_keep]):
                    ev = mybir.InstEventSemaphore(
                        name=f"{inst.name}_xw{j}", ins=[], outs=[])
                    ev.engine = inst.engine
                    ev.sync_info = mybir.SyncInfo(on_wait=[w], on_update=[])
                    out.append(ev)
                inst.sync_info = mybir.SyncInfo(
                    on_wait=waits[-max_keep:], on_update=list(si.on_update))
                changed = True
            out.append(inst)
        if changed:
            blk.instructions = out


def _build_program(legalize=True, sim_compat=False):
    import concourse.bass as bass
    import concourse.tile as tile
    import concourse.mybir as mybir
    from contextlib import ExitStack

    f16 = mybir.dt.float16
    f32 = mybir.dt.float32
    nc = bass.Bass()

    xs = nc.declare_dram_parameter("xs", [S, C, H, W], f16, isOutput=False)
    w2 = nc.declare_dram_parameter("w2", [128, 4, 3, MIP], f16, isOutput=False)
    bias2 = nc.declare_dram_parameter("bias2", [MIP, 1], f32, isOutput=False)
    gwv = nc.declare_dram_parameter("gwv", [MIP, 1], f32, isOutput=False)
    gbv = nc.declare_dram_parameter("gbv", [MIP, 1], f32, isOutput=False)
    wh = nc.declare_dram_parameter("wh", [MIP, C], f16, isOutput=False)
    ww = nc.declare_dram_parameter("ww", [MIP, C], f16, isOutput=False)
    bh = nc.declare_dram_parameter("bh", [128, 2], f32, isOutput=False)
    bw = nc.declare_dram_parameter("bw", [128, 2], f32, isOutput=False)
    out = nc.declare_dram_parameter("out", [S, C, H, W], f16, isOutput=True)

    with tile.TileContext(nc) as tc, ExitStack() as ctx:
        ctx.enter_context(nc.allow_low_precision(reason="2e-2 tolerance, fp16 path"))
        red = nc.vector
        Sig = mybir.ActivationFunctionType.Sigmoid

        singles = ctx.enter_context(tc.tile_pool(name="singles", bufs=1))
        xpool = ctx.enter_context(tc.tile_pool(name="xin", bufs=4))
        ypool = ctx.enter_context(tc.tile_pool(name="yall", bufs=2))
        small = ctx.enter_context(tc.tile_pool(name="small", bufs=4))
        tpool = ctx.enter_context(tc.tile_pool(name="tree", bufs=1))
        apool = ctx.enter_context(tc.tile_pool(name="attn", bufs=4))
        awpool = ctx.enter_context(tc.tile_pool(name="awp", bufs=4))
        pspool = ctx.enter_context(tc.tile_pool(name="ps", bufs=2, space="PSUM"))
        psgate = ctx.enter_context(tc.tile_pool(name="psg", bufs=3, space="PSUM"))

        # ---- x loads on the two HWDGE rings: sample 0 as ring-parallel
        # half-tiles (earliest possible pooling start), the rest as single
        # 2MB transfers alternating rings.  Params go on the idle GPSIMD
        # SWDGE queue so they never queue behind bulk x traffic. ----
        all_xts = []
        for s in range(S):
            xt = xpool.tile([128, 2 * HW], f16, tag="xt")
            all_xts.append(xt)
            for cb in range(2):
                src = xs[s, cb * 128:(cb + 1) * 128].rearrange("c h w -> c (h w)")
                dst = xt[:, cb * HW:(cb + 1) * HW]
                if s == 0 and cb == 0:
                    # quarter loads striped over both rings: first h-half of
                    # the first block lands earliest so pooling starts sooner
                    for q in range(4):
                        eng = nc.sync if q % 2 == 0 else nc.scalar
                        sl = slice(q * HW // 4, (q + 1) * HW // 4)
                        eng.dma_start(out=dst[:, sl], in_=src[:, sl])
                elif s == 0:
                    nc.sync.dma_start(out=dst[:, 0:HW // 2], in_=src[:, 0:HW // 2])
                    nc.scalar.dma_start(out=dst[:, HW // 2:HW], in_=src[:, HW // 2:HW])
                else:
                    eng = nc.sync if cb == 0 else nc.scalar
                    eng.dma_start(out=dst, in_=src)

        w2sb = singles.tile([128, 4, 3, MIP], f16)
        nc.gpsimd.dma_start(out=w2sb, in_=w2[:, :, :, :])
        bias2sb = singles.tile([MIP, 1], f32)
        nc.gpsimd.dma_start(out=bias2sb, in_=bias2[:, :])
        gwsb = singles.tile([MIP, 1], f32)
        nc.gpsimd.dma_start(out=gwsb, in_=gwv[:, :])
        gbsb = singles.tile([MIP, 1], f32)
        nc.gpsimd.dma_start(out=gbsb, in_=gbv[:, :])
        whsb = singles.tile([MIP, C], f16)
        nc.gpsimd.dma_start(out=whsb, in_=wh[:, :])
        wwsb = singles.tile([MIP, C], f16)
        nc.gpsimd.dma_start(out=wwsb, in_=ww[:, :])
        bhsb = singles.tile([128, 2], f32)
        nc.gpsimd.dma_start(out=bhsb, in_=bh[:, :])
        bwsb = singles.tile([128, 2], f32)
        nc.gpsimd.dma_start(out=bwsb, in_=bw[:, :])

        ahxs, aws = [], []
        for s in range(S):
            # ---------- pooling trees ----------
            # Sample 0 is processed per channel-block so pooling starts as
            # soon as the first half-tile lands; later samples use joint ops
            # over both blocks (halved instruction overhead).
            xt = all_xts[s]
            if s == 0:
                y_all = ypool.tile([128, 4, T], f16)
                yrow = y_all.rearrange("p (j c) t -> p j c t", j=2)
                ycol = y_all.rearrange("p (i c) t -> p i c t", i=2)
                # per-channel-block; block 0 additionally per h-half so the
                # first tree runs as soon as the first quarter-MB lands
                def half_trees(xc0, cb, i):
                    xh = xc0[:, i * (HW // 2):(i + 1) * (HW // 2)]
                    rv = xh.rearrange("p (h j w) -> p j h w", j=2, w=W // 2)
                    r1 = tpool.tile([128, 2, H // 2, 16], f16, tag="r1a")
                    red.tensor_add(out=r1, in0=rv[:, :, :, 0:16], in1=rv[:, :, :, 16:32])
                    r2 = tpool.tile([128, 2, H // 2, 8], f16, tag="r2a")
                    red.tensor_add(out=r2, in0=r1[:, :, :, 0:8], in1=r1[:, :, :, 8:16])
                    r3 = tpool.tile([128, 2, H // 2, 4], f16, tag="r3a")
                    red.tensor_add(out=r3, in0=r2[:, :, :, 0:4], in1=r2[:, :, :, 4:8])
                    r4 = tpool.tile([128, 2, H // 2, 2], f16, tag="r4a")
                    red.tensor_add(out=r4, in0=r3[:, :, :, 0:2], in1=r3[:, :, :, 2:4])
                    red.tensor_add(
                        out=yrow[:, :, cb, i * (H // 2):(i + 1) * (H // 2)],
                        in0=r4[:, :, :, 0], in1=r4[:, :, :, 1])
                    cv = xh.rearrange("p (m a w) -> p m a w", a=2, w=W)
                    c1 = tpool.tile([128, 16, W], f16, tag="c1a")
                    red.tensor_add(out=c1, in0=cv[:, :, 0], in1=cv[:, :, 1])
                    cv1 = c1.rearrange("p (m a) w -> p m a w", a=2)
                    c2 = tpool.tile([128, 8, W], f16, tag="c2a")
                    red.tensor_add(out=c2, in0=cv1[:, :, 0], in1=cv1[:, :, 1])
                    cv2 = c2.rearrange("p (m a) w -> p m a w", a=2)
                    c3 = tpool.tile([128, 4, W], f16, tag="c3a")
                    red.tensor_add(out=c3, in0=cv2[:, :, 0], in1=cv2[:, :, 1])
                    cv3 = c3.rearrange("p (m a) w -> p m a w", a=2)
                    c4 = tpool.tile([128, 2, W], f16, tag="c4a")
                    red.tensor_add(out=c4, in0=cv3[:, :, 0], in1=cv3[:, :, 1])
                    red.tensor_add(
                        out=ycol[:, i, cb, H:T],
                        in0=c4[:, 0, :], in1=c4[:, 1, :])

                for i in range(2):
                    half_trees(xt[:, 0:HW], 0, i)
                # block 1: full-block trees (its halves land while block 0
                # is being pooled)
                xc0 = xt[:, HW:2 * HW]
                rv = xc0.rearrange("p (h j w) -> p j h w", j=2, w=W // 2)
                r1 = tpool.tile([128, 2, H, 16], f16, tag="r1b")
                red.tensor_add(out=r1, in0=rv[:, :, :, 0:16], in1=rv[:, :, :, 16:32])
                r2 = tpool.tile([128, 2, H, 8], f16, tag="r2b")
                red.tensor_add(out=r2, in0=r1[:, :, :, 0:8], in1=r1[:, :, :, 8:16])
                r3 = tpool.tile([128, 2, H, 4], f16, tag="r3b")
                red.tensor_add(out=r3, in0=r2[:, :, :, 0:4], in1=r2[:, :, :, 4:8])
                r4 = tpool.tile([128, 2, H, 2], f16, tag="r4b")
                red.tensor_add(out=r4, in0=r3[:, :, :, 0:2], in1=r3[:, :, :, 2:4])
                red.tensor_add(
                    out=yrow[:, :, 1, 0:H],
                    in0=r4[:, :, :, 0], in1=r4[:, :, :, 1])
                cv = xc0.rearrange("p (i m a w) -> p i m a w", i=2, a=2, w=W)
                c1 = tpool.tile([128, 2, 16, W], f16, tag="c1b")
                red.tensor_add(out=c1, in0=cv[:, :, :, 0], in1=cv[:, :, :, 1])
                cv1 = c1.rearrange("p i (m a) w -> p i m a w", a=2)
                c2 = tpool.tile([128, 2, 8, W], f16, tag="c2b")
                red.tensor_add(out=c2, in0=cv1[:, :, :, 0], in1=cv1[:, :, :, 1])
                cv2 = c2.rearrange("p i (m a) w -> p i m a w", a=2)
                c3 = tpool.tile([128, 2, 4, W], f16, tag="c3b")
                red.tensor_add(out=c3, in0=cv2[:, :, :, 0], in1=cv2[:, :, :, 1])
                cv3 = c3.rearrange("p i (m a) w -> p i m a w", a=2)
                c4 = tpool.tile([128, 2, 2, W], f16, tag="c4b")
                red.tensor_add(out=c4, in0=cv3[:, :, :, 0], in1=cv3[:, :, :, 1])
                red.tensor_add(
                    out=ycol[:, :, 1, H:T],
                    in0=c4[:, :, 0, :], in1=c4[:, :, 1, :])
            else:
                y_all = ypool.tile([128, 4, T], f16)
                # row-half sums: w-quarter tree; (h, j) folds into one
                # stride-32 dim so joint ops stay within 3 free dims
                rv = xt.rearrange("p (cb hj w) -> p cb hj w", cb=2, w=W // 2)
                r1 = tpool.tile([128, 2, 2 * H, 16], f16, tag="r1")
                red.tensor_add(out=r1, in0=rv[:, :, :, 0:16], in1=rv[:, :, :, 16:32])
                r2 = tpool.tile([128, 2, 2 * H, 8], f16, tag="r2")
                red.tensor_add(out=r2, in0=r1[:, :, :, 0:8], in1=r1[:, :, :, 8:16])
                r3 = tpool.tile([128, 2, 2 * H, 4], f16, tag="r3")
                red.tensor_add(out=r3, in0=r2[:, :, :, 0:4], in1=r2[:, :, :, 4:8])
                r4 = tpool.tile([128, 2, 2 * H, 2], f16, tag="r4")
                red.tensor_add(out=r4, in0=r3[:, :, :, 0:2], in1=r3[:, :, :, 2:4])
                rv4 = r4.rearrange("p cb (h j) a -> p cb j h a", j=2)
                red.tensor_add(
                    out=y_all.rearrange("p (j c) t -> p c j t", j=2)[:, :, :, 0:H],
                    in0=rv4[:, :, :, :, 0], in1=rv4[:, :, :, :, 1])
                # col-half sums: row-pair tree ((cb, i) folds to stride 2048)
                cv = xt.rearrange("p (ci m a w) -> p ci m a w", ci=4, a=2, w=W)
                c1 = tpool.tile([128, 4, 16, W], f16, tag="c1")
                red.tensor_add(out=c1, in0=cv[:, :, :, 0], in1=cv[:, :, :, 1])
                cv1 = c1.rearrange("p ci (m a) w -> p ci m a w", a=2)
                c2 = tpool.tile([128, 4, 8, W], f16, tag="c2")
                red.tensor_add(out=c2, in0=cv1[:, :, :, 0], in1=cv1[:, :, :, 1])
                cv2 = c2.rearrange("p ci (m a) w -> p ci m a w", a=2)
                c3 = tpool.tile([128, 4, 4, W], f16, tag="c3")
                red.tensor_add(out=c3, in0=cv2[:, :, :, 0], in1=cv2[:, :, :, 1])
                cv3 = c3.rearrange("p ci (m a) w -> p ci m a w", a=2)
                c4 = tpool.tile([128, 4, 2, W], f16, tag="c4")
                red.tensor_add(out=c4, in0=cv3[:, :, :, 0], in1=cv3[:, :, :, 1])
                cv4 = c4.rearrange("p (cb i) a w -> p cb i a w", cb=2)
                red.tensor_add(
                    out=y_all.rearrange("p (i c) t -> p c i t", i=2)[:, :, :, H:T],
                    in0=cv4[:, :, :, 0, :], in1=cv4[:, :, :, 1, :])

            # ---------- dilated conv as 12 accumulated matmuls ----------
            psy = pspool.tile([MIP, T], f32, tag="psy")
            order = [(0, 1)] + [(g, k) for g in range(4) for k in range(3) if (g, k) != (0, 1)]
            for idx, (g, k) in enumerate(order):
                lhsT = w2sb[:, g, k, :]
                if k == 1:
                    o_sl, i_sl = slice(0, T), slice(0, T)
                elif k == 0:
                    o_sl, i_sl = slice(2, T), slice(0, T - 2)
                else:
                    o_sl, i_sl = slice(0, T - 2), slice(2, T)
                nc.tensor.matmul(
                    out=psy[:, o_sl],
                    lhsT=lhsT,
                    rhs=y_all[:, g, i_sl],
                    start=(idx == 0),
                    stop=(idx == len(order) - 1),
                )

            # ---------- bias + BN (folded) + SiLU + SE (sigmoid-only ACT) ----
            ya0 = small.tile([MIP, T], f32, tag="ya0")
            nc.vector.tensor_scalar_add(out=ya0, in0=psy, scalar1=bias2sb[:, :])
            ysg = small.tile([MIP, T], f32, tag="ysg")
            nc.scalar.activation(out=ysg, in_=ya0, func=Sig, bias=0.0, scale=1.0)
            ya = small.tile([MIP, T], f32, tag="ya")
            red.tensor_mul(out=ya, in0=ya0, in1=ysg)
            ysum = small.tile([MIP, 1], f32, tag="ysum")
            red.reduce_sum(out=ysum, in_=ya, axis=mybir.AxisListType.X)
            se = small.tile([MIP, 1], f32, tag="se")
            nc.scalar.activation(out=se, in_=ysum, func=Sig,
                                 bias=gbsb[:, :], scale=gwsb[:, :])
            yg = small.tile([MIP, T], f16, tag="yg")
            nc.vector.tensor_scalar_mul(out=yg, in0=ya, scalar1=se[:, :])

            # ---------- h/w attention gates (PE + ACT only; muls deferred) ---
            ahx2 = apool.tile([128, 2, H, W], f16, tag="ahx")
            aw2 = awpool.tile([128, 2, W], f16, tag="aw")
            ahxs.append(ahx2)
            aws.append(aw2)
            for cb in range(2):
                psa = psgate.tile([128, H], f32, tag="psa")
                nc.tensor.matmul(
                    out=psa,
                    lhsT=whsb[:, cb * 128:(cb + 1) * 128],
                    rhs=yg[:, 0:H], start=True, stop=True,
                )
                # fused sigmoid + broadcast-expand on ACT: ahx[p,h,w]=sig(psa[p,h]+bh)
                pa = psa[:, :]
                pab = bass.AP(tensor=pa.tensor, offset=pa.offset,
                              ap=[pa.ap[0], pa.ap[1], [0, W]])
                nc.scalar.activation(out=ahx2[:, cb], in_=pab, func=Sig,
                                     bias=bhsb[:, cb:cb + 1], scale=1.0)
                psb = psgate.tile([128, W], f32, tag="psb")
                nc.tensor.matmul(
                    out=psb,
                    lhsT=wwsb[:, cb * 128:(cb + 1) * 128],
                    rhs=yg[:, H:T], start=True, stop=True,
                )
                nc.scalar.activation(out=aw2[:, cb], in_=psb, func=Sig,
                                     bias=bwsb[:, cb:cb + 1], scale=1.0)

        # ---------- final multiplies (both 2x on DVE, in place) + stores -----
        for s in range(S):
            xt = all_xts[s]
            xv = xt.rearrange("p (ch w) -> p ch w", w=W)
            av = ahxs[s].rearrange("p cb h w -> p (cb h) w")
            red.tensor_mul(out=xv, in0=xv, in1=av)
            a = aws[s][:, :, :]
            awb = bass.AP(tensor=a.tensor, offset=a.offset,
                          ap=[a.ap[0], a.ap[1], [0, H], a.ap[2]])
            xc = xt.rearrange("p (cb h w) -> p cb h w", cb=2, w=W)
            ost = out[s].rearrange("(cb c) h w -> c cb (h w)", cb=2)
            if s < S - 1:
                red.tensor_mul(out=xc, in0=xc, in1=awb)
                # store halves on both rings so no single ring backs up
                nc.sync.dma_start(out=ost[:, 0], in_=xt[:, 0:HW])
                nc.scalar.dma_start(out=ost[:, 1], in_=xt[:, HW:2 * HW])
            else:
                # last sample: mul2 in h-half chunks with eighth-tile stores
                # striped over both rings so the final transfer is tiny
                for cb in range(2):
                    acb = aws[s][:, cb, :]
                    for hh in range(2):
                        hsl = slice(hh * (H // 2), (hh + 1) * (H // 2))
                        red.tensor_mul(
                            out=xc[:, cb, hsl], in0=xc[:, cb, hsl],
                            in1=bass.AP(tensor=acb.tensor, offset=acb.offset,
                                        ap=[acb.ap[0], [0, H // 2], acb.ap[1]]))
                        for qq in range(2):
                            eng = nc.sync if qq == 0 else nc.scalar
                            lo = cb * HW + hh * (HW // 2) + qq * (HW // 4)
                            osl = slice(hh * (HW // 2) + qq * (HW // 4),
                                        hh * (HW // 2) + (qq + 1) * (HW // 4))
                            eng.dma_start(out=ost[:, cb, osl],
                                          in_=xt[:, lo:lo + HW // 4])
    if legalize:
        _legalize_waits(nc, mybir)
    return nc


def _prep_params(conv1_w, conv1_b, bn_gamma, bn_beta, bn_mean, bn_var,
                 gate_w, gate_b, convh_w, convh_b, convw_w, convw_b):
    f32 = np.float32
    bnscale = (np.asarray(bn_gamma, f32)
               / np.sqrt(np.asarray(bn_var, f32) + BN_EPS)).astype(f32)
    Wc = np.asarray(conv1_w, f32)[:, :, :, 1]                     # [MIP, 768, 3]
    s_ci = np.where(np.arange(3 * C) < C, 1.0 / W, 2.0 / W).astype(f32)
    W2 = (Wc * s_ci[None, :, None] * bnscale[:, None, None]).astype(f32)
    bias2 = ((np.asarray(conv1_b, f32) - np.asarray(bn_mean, f32)) * bnscale
             + np.asarray(bn_beta, f32)).astype(f32)
    # Fold the full-mean channel blocks (g=0,1) into the four half-sum
    # blocks: conv is linear and fullsum = half0sum + half1sum, so
    # W'[g'] = W[2+g'] + W[g'%2] and only 4 channel-blocks remain.
    W6 = W2.reshape(MIP, 6, 128, 3)
    W4 = np.stack([W6[:, 2 + gp] + W6[:, gp % 2] for gp in range(4)], axis=1)
    # w2 layout [ci_local=128, g'=4, k=3, o=MIP]
    w2 = np.ascontiguousarray(W4.transpose(2, 1, 3, 0)).astype(np.float16)
    gw = np.full((MIP, 1), float(gate_w) / T, f32)
    gb = np.full((MIP, 1), float(gate_b), f32)
    wh = np.ascontiguousarray(np.asarray(convh_w, np.float16).T)   # [MIP, 256]
    ww = np.ascontiguousarray(np.asarray(convw_w, np.float16).T)
    bh = np.ascontiguousarray(np.asarray(convh_b, f32).reshape(2, 128).T)  # [128, 2]
    bw = np.ascontiguousarray(np.asarray(convw_b, f32).reshape(2, 128).T)
    return dict(w2=w2, bias2=bias2.reshape(MIP, 1), gwv=gw, gbv=gb,
                wh=wh, ww=ww, bh=bh, bw=bw)


def kernel(**inputs):
    import sys
    if "/opt/trn_rl_repo" not in sys.path:
        sys.path.insert(0, "/opt/trn_rl_repo")
    from concourse.bass_utils import run_bass_kernel_spmd

    x = np.asarray(inputs["x"], np.float32).astype(np.float16)
    params = _prep_params(
        inputs["conv1_w"], inputs["conv1_b"], inputs["bn_gamma"],
        inputs["bn_beta"], inputs["bn_mean"], inputs["bn_var"],
        inputs["gate_w"], inputs["gate_b"], inputs["convh_w"],
        inputs["convh_b"], inputs["convw_w"], inputs["convw_b"])

    if "nc" not in _CACHE:
        _CACHE["nc"] = _build_program()
    nc = _CACHE["nc"]

    in_maps = [
        {"xs": np.ascontiguousarray(x[i * S:(i + 1) * S]), **params}
        for i in range(N_CORES)
    ]
    res = run_bass_kernel_spmd(nc, in_maps, core_ids=list(range(N_CORES)))
    out = np.concatenate([r["out"] for r in res.results], axis=0)
    return out.astype(np.float32)
